# revision 29
# baseline (speedup 1.0000x reference)
"""Trainium2 Bass kernel for the Tolles-Lawson custom loss.

reference:
    c = model_output[:, :18]; d = model_output[:, 18:19]
    tmp = sum(A * (beta_TL + c), axis=1, keepdims=True) + d
    L = mean((tmp - y)^2) + mean((tmp - B_tl)^2)

Sharding: pure data parallel over rows on 8 cores (R = 524,288 rows per
core, tail zero-padded; zero rows contribute 0 to both sums). Per-core
partial sums land in accs [128, 18]; the all-reduce is host-side.

Layout (per core), "transposed": rows split into 64 groups of NCG=8192;
SBUF partition p = 2*g + s holds slot s (coefficients 9s..9s+8) of
group g, rows along the free axis as 9 slabs of 512 columns per chunk.
The 18-way row-reduction runs on the TensorEngine: for each 512-column
chunk, 9 accumulating matmuls (one per coefficient slab t) with a
block-ones stationary W1 [128, 64] (W1[2g+s, g] = 1) compute
    psum[64b+g, n] = sum_s sum_t prod[2g+s, (b,t,n)]
(b = chunk parity selecting the PSUM base partition 0/64 - matmul
output base must be 0/32/64). The beta term sum_j A_ij * beta_j is
folded into 9 more matmuls with W_bt[2g+s, g] = beta[9s+t] applied to
the raw A stream, so no engine ever materializes (c + beta).
Two chunks fill a fat [128, 512] PSUM tile; the epilogue runs at full
partition width.

Engine split per fill [128, 9216] (= 65,536 rows):
  ACT : fp8->bf16 upcast of c (Copy), squares+accum of e1/e2
  DVE : fp8->bf16 upcast of a (first XF8 fills), prod = a * c (2x mode),
        e0 = psum + d, e1/e2 = e0 - y/b
  PE  : 2 blocks x (9 beta-matmuls on a + 9 ones-matmuls on prod)
  DMA : a [128,9216] (fp8 first XF8 fills, bf16 after), c fp8(e4m3),
        dyb [128,1536] bf16

Precision: c is always fp8 e4m3, a is fp8 on the first XF8 fills
(~27 MB/core/pass vs 39 MB bf16 baseline; measured DMA is ~326 GB/s so
the floor is ~82 us). The per-element quantization noise is random and
washes out in the 4M-row mean (measured rel err ~1e-3 vs 2e-2 budget).

Drain taper: the benchmark (and any single launch) pays the pipeline
drain after the last DMA. The last fill is bf16 (no upcast in its
chain), its c/dyb are prefetched early, and it is processed as two
half-fills of one 512-column chunk each so the post-DMA tail is a short
[128,4608] multiply + 9 matmuls + [64,512] epilogue.
"""

import numpy as np
import ml_dtypes

import concourse.bacc as bacc
import concourse.mybir as mybir
from concourse import tile
from concourse.bass_utils import run_bass_kernel_spmd

N_TOTAL = 4_000_000
NCOEF = 18
NG = 64                # row groups per core
NSLOT = 2              # coefficient slots per group
NSLAB = 9              # coefficients per slot
P = NG * NSLOT         # 128 SBUF partitions
F = 512                # chunk columns (one PSUM bank of f32)
NBLK = 2               # chunks per fill -> NBLK*NG = 128 psum partitions
HFILL = F * NSLAB      # 4608 free elements per half-fill (one chunk)
FILLW = HFILL * NBLK   # 9216 free elements per fill per partition
NFILL = 8              # fills per core per pass
NCHUNK = NBLK * NFILL  # 16 chunks per group
NCG = F * NCHUNK       # 8192 columns per group
R = NG * NCG           # 524288 rows per core
N_CORES = 8

XF8 = 4                # number of fills whose a-stream is fp8
TAPER = True           # process the last fill as two half-fills
SQMOVE = 0             # bf16-a fills whose squares run on DVE, not ACT
                       # (tensor_tensor_reduce trips a walrus codegen error
                       # on this stack; keep 0)


def fp8_fills(xf8):
    """Spread the fp8-a fills among fills 0..NFILL-2 (the taper fill is
    always bf16) so heavy upcast fills alternate with light ones."""
    if xf8 <= 0:
        return []
    return sorted(set(
        int(round(i * (NFILL - 2) / max(xf8 - 1, 1))) for i in range(xf8)))

f32 = mybir.dt.float32
bf16 = mybir.dt.bfloat16
fp8 = mybir.dt.float8e4
add = mybir.AluOpType.add
sub = mybir.AluOpType.subtract
mult = mybir.AluOpType.mult
COPY = mybir.ActivationFunctionType.Copy
SQ = mybir.ActivationFunctionType.Square

_cached = {}


def _build(hw_rep=0, dma_only=False, probe=None, xf8=XF8, taper=TAPER,
           sqmove=SQMOVE):
    """hw_rep > 0 wraps the pass in a For_i hardware loop (bench only).

    probe: None = full kernel; "dma" = DMAs only.
    """
    if dma_only:
        probe = "dma"
    key = (hw_rep, probe, xf8, taper, sqmove)
    if key in _cached:
        return _cached[key]
    f8set = set(fp8_fills(xf8))
    sq_dve_set = set(
        [f for f in range(NFILL - 1) if f not in f8set][:sqmove])
    a8_slot = {f: i for i, f in enumerate(sorted(f8set))}
    a16_slot = {f: i for i, f in
                enumerate(f for f in range(NFILL) if f not in f8set)}

    nc = bacc.Bacc(None)
    a8_ext = c8_ext = a16_ext = None
    if xf8 > 0:
        a8_ext = nc.declare_dram_parameter("a8", [P, xf8 * FILLW], fp8,
                                           isOutput=False)
    if xf8 < NFILL:
        a16_ext = nc.declare_dram_parameter(
            "a16", [P, (NFILL - xf8) * FILLW], bf16, isOutput=False)
    c8_ext = nc.declare_dram_parameter("c8", [P, NFILL * FILLW], fp8,
                                       isOutput=False)
    dyb_ext = nc.declare_dram_parameter("dyb", [P, NFILL * 3 * F], bf16,
                                        isOutput=False)
    # wb[:, 64*t : 64*(t+1)] = W_beta_t for t < 9; wb[:, 576:640] = W1
    wb_ext = nc.declare_dram_parameter("wb", [P, (NSLAB + 1) * NG], bf16,
                                       isOutput=False)
    out_ext = nc.declare_dram_parameter("out", [P, 2 * (NFILL + 1)], f32,
                                        isOutput=True)

    with tile.TileContext(nc) as tc:
        with tc.tile_pool(name="consts", bufs=1) as consts, \
             tc.tile_pool(name="a8io", bufs=2) as a8io, \
             tc.tile_pool(name="aio", bufs=3) as aio, \
             tc.tile_pool(name="ahio", bufs=2) as ahio, \
             tc.tile_pool(name="cio", bufs=4) as cio, \
             tc.tile_pool(name="dio", bufs=4) as dio, \
             tc.tile_pool(name="work", bufs=3) as work, \
             tc.tile_pool(name="epi", bufs=2) as epi, \
             tc.psum_pool(name="ps", bufs=6) as pspool:
            wb_t = consts.tile([P, (NSLAB + 1) * NG], bf16, name="wb",
                               tag="wb")
            nc.sync.dma_start(out=wb_t[:], in_=wb_ext[:])
            accs = consts.tile([P, 2 * (NFILL + 1)], f32, name="accs",
                               tag="accs")
            nc.vector.memset(accs[:], 0.0)

            def wview(t):
                return wb_t[:, NG * t:NG * (t + 1)]

            def dma_a(f):
                if f in f8set:
                    a_ch = a8io.tile([P, FILLW], fp8, tag="a8", name=f"a{f}")
                    fo = a8_slot[f]
                    src = a8_ext[:, fo * FILLW:(fo + 1) * FILLW]
                else:
                    a_ch = aio.tile([P, FILLW], bf16, tag="a16", name=f"a{f}")
                    fo = a16_slot[f]
                    src = a16_ext[:, fo * FILLW:(fo + 1) * FILLW]
                nc.sync.dma_start(out=a_ch[:], in_=src)
                return a_ch

            def dma_a_half(f, h):
                fo = a16_slot[f]
                a_ch = ahio.tile([P, HFILL], bf16, tag="a16h",
                                 name=f"a{f}h{h}")
                lo = fo * FILLW + h * HFILL
                nc.sync.dma_start(out=a_ch[:], in_=a16_ext[:, lo:lo + HFILL])
                return a_ch

            def dma_c(f):
                c_ch = cio.tile([P, FILLW], fp8, tag="c", name=f"c{f}")
                nc.sync.dma_start(out=c_ch[:],
                                  in_=c8_ext[:, f * FILLW:(f + 1) * FILLW])
                return c_ch

            def dma_dyb(f):
                dyb_ch = dio.tile([P, 3 * F], bf16, tag="dyb", name=f"dyb{f}")
                nc.sync.dma_start(
                    out=dyb_ch[:], in_=dyb_ext[:, f * 3 * F:(f + 1) * 3 * F])
                return dyb_ch

            def epilogue(slot, ps, dyb_ch, p0, np_, sq_dve=False):
                """Square-accumulate (psum+d-y) and (psum+d-b) for psum
                partitions [p0, p0+np_) into accs column pair `slot`.
                sq_dve: run the squares on DVE (tensor_tensor_reduce)
                instead of ACT, to balance engine load."""
                sl = slice(p0, p0 + np_)
                pv = ps[sl, :]
                d_v = dyb_ch[sl, 0:F]
                y_v = dyb_ch[sl, F:2 * F]
                b_v = dyb_ch[sl, 2 * F:3 * F]
                e0 = epi.tile([P, F], bf16, tag="e0", name=f"e0_{slot}")
                nc.vector.tensor_tensor(e0[sl, :], pv, d_v, add)
                e1 = epi.tile([P, F], bf16, tag="e1", name=f"e1_{slot}")
                nc.vector.tensor_tensor(e1[sl, :], e0[sl, :], y_v, sub)
                e2 = epi.tile([P, F], bf16, tag="e2", name=f"e2_{slot}")
                nc.vector.tensor_tensor(e2[sl, :], e0[sl, :], b_v, sub)
                s1 = epi.tile([P, F], bf16, tag="s1", name=f"s1_{slot}")
                s2 = epi.tile([P, F], bf16, tag="s2", name=f"s2_{slot}")
                a1 = accs[sl, 2 * slot:2 * slot + 1]
                a2 = accs[sl, 2 * slot + 1:2 * slot + 2]
                if sq_dve:
                    nc.vector.tensor_tensor_reduce(
                        s1[sl, :], e1[sl, :], e1[sl, :], 1.0, 0.0,
                        mult, add, a1)
                    nc.vector.tensor_tensor_reduce(
                        s2[sl, :], e2[sl, :], e2[sl, :], 1.0, 0.0,
                        mult, add, a2)
                else:
                    nc.scalar.activation(s1[sl, :], e1[sl, :], SQ,
                                         accum_out=a1)
                    nc.scalar.activation(s2[sl, :], e2[sl, :], SQ,
                                         accum_out=a2)

            def matmuls(out_v, a_view, prod_view):
                # beta matmuls first: they only need the a stream, so PE
                # starts while DVE is still computing prod
                for t in range(NSLAB):
                    nc.tensor.matmul(out_v, wview(t),
                                     a_view[:, t * F:(t + 1) * F],
                                     start=(t == 0), stop=False)
                for t in range(NSLAB):
                    nc.tensor.matmul(out_v, wview(NSLAB),
                                     prod_view[:, t * F:(t + 1) * F],
                                     start=False, stop=(t == NSLAB - 1))

            def half_work(f, b, a_v, a_is_fp8, c_v, ps_v):
                """One chunk: c upcast, multiply, 18 matmuls into ps_v.
                An fp8 a-stream is read directly by both DVE (the multiply
                drops to 1x mode, same cost as upcast+2x multiply but one
                fewer op/dependency) and PE."""
                bc = work.tile([P, HFILL], bf16, tag="bc", name=f"bc{f}_{b}")
                nc.scalar.activation(bc[:], c_v, COPY)
                prod = work.tile([P, HFILL], bf16, tag="prod",
                                 name=f"p{f}_{b}")
                nc.vector.tensor_tensor(prod[:], a_v, bc[:], mult)
                matmuls(ps_v, a_v, prod[:])

            def fill_work(f, a_ch, c_ch):
                ps = pspool.tile([P, F], f32, tag="ps", name=f"ps{f}")
                for b in range(NBLK):
                    base = b * HFILL
                    half_work(f, b, a_ch[:, base:base + HFILL], f in f8set,
                              c_ch[:, base:base + HFILL],
                              ps[NG * b:NG * (b + 1), :])
                return ps

            def body():
                n_main = NFILL - 1 if taper else NFILL
                pend = []  # software pipeline: epilogue(f) after work(f+1)
                c_last = dyb_last = None
                for f in range(n_main):
                    a_ch = dma_a(f)
                    c_ch = dma_c(f)
                    dyb_ch = dma_dyb(f)
                    if f == n_main - 2 and taper:
                        # prefetch the taper fill's small streams so its
                        # a-halves are the only post-prefetch DMAs
                        c_last = dma_c(NFILL - 1)
                        dyb_last = dma_dyb(NFILL - 1)
                    if probe == "dma":
                        continue
                    ps = fill_work(f, a_ch, c_ch)
                    pend.append((f, ps, dyb_ch))
                    if len(pend) > 1:
                        fo, pso, dybo = pend.pop(0)
                        epilogue(fo, pso, dybo, 0, P, fo in sq_dve_set)
                if taper:
                    fl = NFILL - 1
                    if probe == "dma":
                        for h in range(NBLK):
                            dma_a_half(fl, h)
                    else:
                        psl = pspool.tile([P, F], f32, tag="ps", name="psl")
                        for h in range(NBLK):
                            a_h = dma_a_half(fl, h)
                            half_work(fl, h, a_h[:], False,
                                      c_last[:, h * HFILL:(h + 1) * HFILL],
                                      psl[NG * h:NG * (h + 1), :])
                            if pend:
                                fo, pso, dybo = pend.pop(0)
                                epilogue(fo, pso, dybo, 0, P,
                                         fo in sq_dve_set)
                            epilogue(fl + h, psl, dyb_last, NG * h, NG)
                for args in pend:
                    f, pso, dybo = args
                    epilogue(f, pso, dybo, 0, P, f in sq_dve_set)

            if hw_rep:
                with tc.For_i(0, hw_rep) as _:
                    body()
            else:
                body()

            nc.sync.dma_start(out=out_ext[:], in_=accs[:])
    nc.finalize()
    _cached[key] = nc
    return nc


def _prepare_in_maps(model_output, y, A, B_tl, beta_TL, xf8=XF8):
    model_output = np.asarray(model_output, dtype=np.float32)
    y = np.asarray(y, dtype=np.float32)
    A = np.asarray(A, dtype=np.float32)
    B_tl = np.asarray(B_tl, dtype=np.float32)
    beta_TL = np.asarray(beta_TL, dtype=np.float32)

    # wb: 9 beta-weight matrices then the ones matrix, each [128, 64]
    wb = np.zeros((P, (NSLAB + 1) * NG), dtype=np.float32)
    g_idx = np.arange(NG)
    for t in range(NSLAB):
        for s in range(NSLOT):
            wb[NSLOT * g_idx + s, NG * t + g_idx] = beta_TL[NSLAB * s + t]
    for s in range(NSLOT):
        wb[NSLOT * g_idx + s, NG * NSLAB + g_idx] = 1.0
    wb = wb.astype(ml_dtypes.bfloat16)

    def tcoef(arr):
        # [R, 18] -> partition 2g+s, free (chunk, slab t, col n), coef 9s+t
        return np.ascontiguousarray(
            arr.reshape(NG, NCHUNK, F, NSLOT, NSLAB)
            .transpose(0, 3, 1, 4, 2).reshape(P, NFILL * FILLW))

    def tfat(arr):
        # [R] -> [NFILL, 128(=64b+g), F]
        return arr.reshape(NG, NFILL, NBLK, F).transpose(1, 2, 0, 3) \
                  .reshape(NFILL, P, F)

    in_maps = []
    for i in range(N_CORES):
        lo, hi = i * R, min((i + 1) * R, N_TOTAL)
        mo_sh = np.zeros((R, NCOEF + 1), dtype=np.float32)
        mo_sh[:hi - lo] = model_output[lo:hi]
        a_sh = np.zeros((R, NCOEF), dtype=np.float32)
        a_sh[:hi - lo] = A[lo:hi]
        d_sh = mo_sh[:, NCOEF].copy()
        y_sh = np.zeros((R,), dtype=np.float32)
        y_sh[:hi - lo] = y[lo:hi, 0]
        b_sh = np.zeros((R,), dtype=np.float32)
        b_sh[:hi - lo] = B_tl[lo:hi, 0]

        a_t = tcoef(a_sh)
        c_t = tcoef(mo_sh[:, :NCOEF]).astype(ml_dtypes.float8_e4m3)
        dyb = np.ascontiguousarray(
            np.concatenate([tfat(d_sh), tfat(y_sh), tfat(b_sh)], axis=2)
            .transpose(1, 0, 2)
            .reshape(P, NFILL * 3 * F)).astype(ml_dtypes.bfloat16)
        m = {"c8": c_t, "dyb": dyb, "wb": wb}
        f8 = fp8_fills(xf8)
        f16 = [f for f in range(NFILL) if f not in f8]
        af = a_t.reshape(P, NFILL, FILLW)
        if f8:
            m["a8"] = np.ascontiguousarray(
                af[:, f8, :].reshape(P, -1)).astype(ml_dtypes.float8_e4m3)
        if f16:
            m["a16"] = np.ascontiguousarray(
                af[:, f16, :].reshape(P, -1)).astype(ml_dtypes.bfloat16)
        in_maps.append(m)
    return in_maps


def kernel(model_output, y, A, B_tl, beta_TL):
    nc = _build()
    in_maps = _prepare_in_maps(model_output, y, A, B_tl, beta_TL)
    res = run_bass_kernel_spmd(nc, in_maps, list(range(N_CORES)))
    total = 0.0
    for r in res.results:
        total += float(r["out"].astype(np.float64).sum())
    return np.asarray(total / N_TOTAL, dtype=np.float32)


# revision 32
# speedup vs baseline: 1.0371x; 1.0371x over previous
"""Trainium2 Bass kernel for the Tolles-Lawson custom loss.

reference:
    c = model_output[:, :18]; d = model_output[:, 18:19]
    tmp = sum(A * (beta_TL + c), axis=1, keepdims=True) + d
    L = mean((tmp - y)^2) + mean((tmp - B_tl)^2)

Sharding: pure data parallel over rows on 8 cores (R = 524,288 rows per
core, tail zero-padded; zero rows contribute 0 to both sums). Per-core
partial sums land in accs [128, 18]; the all-reduce is host-side.

Layout (per core), "transposed": rows split into 64 groups of NCG=8192;
SBUF partition p = 2*g + s holds slot s (coefficients 9s..9s+8) of
group g, rows along the free axis as 9 slabs of 512 columns per chunk.
The 18-way row-reduction runs on the TensorEngine: for each 512-column
chunk, 9 accumulating matmuls (one per coefficient slab t) with a
block-ones stationary W1 [128, 64] (W1[2g+s, g] = 1) compute
    psum[64b+g, n] = sum_s sum_t prod[2g+s, (b,t,n)]
(b = chunk parity selecting the PSUM base partition 0/64 - matmul
output base must be 0/32/64). The beta term sum_j A_ij * beta_j is
folded into 9 more matmuls with W_bt[2g+s, g] = beta[9s+t] applied to
the raw A stream, so no engine ever materializes (c + beta).
Two chunks fill a fat [128, 512] PSUM tile; the epilogue runs at full
partition width.

Engine split per fill [128, 9216] (= 65,536 rows):
  ACT : fp8->bf16 upcast of c (Copy), squares+accum of e1/e2
  DVE : fp8->bf16 upcast of a (first XF8 fills), prod = a * c (2x mode),
        e0 = psum + d, e1/e2 = e0 - y/b
  PE  : 2 blocks x (9 beta-matmuls on a + 9 ones-matmuls on prod)
  DMA : a [128,9216] (fp8 first XF8 fills, bf16 after), c fp8(e4m3),
        dyb [128,1536] bf16

Precision: c is always fp8 e4m3, a is fp8 on XF8=4 of 8 fills,
interleaved with bf16-a fills so per-fill engine load stays under the
DMA cadence (~26.8 MB/core/pass vs 39 MB bf16 baseline). The
per-element quantization noise is random and washes out in the 4M-row
mean (measured rel err 1.1e-3 vs the 2e-2 budget).

Drain taper: each measured pass (and any single launch) pays the
pipeline drain after the last DMA. The last fill is bf16 (no upcast in
its chain), its c/dyb are prefetched early, and it is processed as two
half-fills of one 512-column chunk each so the post-DMA tail is a short
[128,4608] multiply + 18 matmuls + [64,512] epilogue.

Measured (For_i slope, 8 cores): 102.9 us/pass vs 151.8 us for the
previous row-major DVE-tree kernel and 96.9->81.1 us for the DMA
streams alone (i.e. ~326-330 GB/s/core effective, 91% of the 358 GB/s
HBM-per-core limit; the kernel runs ~22 us above its DMA floor, the
residual being For_i's per-iteration all-engine barrier drain plus
cross-engine dependency stalls). Engine busy estimates per pass:
DMA 81 us, ACT ~79 us (upcasts + squares), PE ~77 us (288 self-loading
matmuls), DVE ~72 us (upcasts, multiplies, epilogue).
"""

import numpy as np
import ml_dtypes

import concourse.bacc as bacc
import concourse.mybir as mybir
from concourse import tile
from concourse.bass_utils import run_bass_kernel_spmd

N_TOTAL = 4_000_000
NCOEF = 18
NG = 64                # row groups per core
NSLOT = 2              # coefficient slots per group
NSLAB = 9              # coefficients per slot
P = NG * NSLOT         # 128 SBUF partitions
F = 512                # chunk columns (one PSUM bank of f32)
NBLK = 2               # chunks per fill -> NBLK*NG = 128 psum partitions
HFILL = F * NSLAB      # 4608 free elements per half-fill (one chunk)
FILLW = HFILL * NBLK   # 9216 free elements per fill per partition
NFILL = 8              # fills per core per pass
NCHUNK = NBLK * NFILL  # 16 chunks per group
NCG = F * NCHUNK       # 8192 columns per group
R = NG * NCG           # 524288 rows per core
N_CORES = 8

XF8 = 4                # number of fills whose a-stream is fp8
TAPER = True           # process the last fill as two half-fills
SQMOVE = 0             # bf16-a fills whose squares run on DVE, not ACT
                       # (tensor_tensor_reduce trips a walrus codegen error
                       # on this stack; keep 0)


def fp8_fills(xf8):
    """Spread the fp8-a fills among fills 0..NFILL-2 (the taper fill is
    always bf16) so heavy upcast fills alternate with light ones."""
    if xf8 <= 0:
        return []
    return sorted(set(
        int(round(i * (NFILL - 2) / max(xf8 - 1, 1))) for i in range(xf8)))

f32 = mybir.dt.float32
bf16 = mybir.dt.bfloat16
fp8 = mybir.dt.float8e4
add = mybir.AluOpType.add
sub = mybir.AluOpType.subtract
mult = mybir.AluOpType.mult
COPY = mybir.ActivationFunctionType.Copy
SQ = mybir.ActivationFunctionType.Square

_cached = {}


def _build(hw_rep=0, dma_only=False, probe=None, xf8=XF8, taper=TAPER,
           sqmove=SQMOVE):
    """hw_rep > 0 wraps the pass in a For_i hardware loop (bench only).

    probe: None = full kernel; "dma" = DMAs only.
    """
    if dma_only:
        probe = "dma"
    key = (hw_rep, probe, xf8, taper, sqmove)
    if key in _cached:
        return _cached[key]
    f8set = set(fp8_fills(xf8))
    sq_dve_set = set(
        [f for f in range(NFILL - 1) if f not in f8set][:sqmove])
    a8_slot = {f: i for i, f in enumerate(sorted(f8set))}
    a16_slot = {f: i for i, f in
                enumerate(f for f in range(NFILL) if f not in f8set)}

    nc = bacc.Bacc(None)
    a8_ext = c8_ext = a16_ext = None
    if xf8 > 0:
        a8_ext = nc.declare_dram_parameter("a8", [P, xf8 * FILLW], fp8,
                                           isOutput=False)
    if xf8 < NFILL:
        a16_ext = nc.declare_dram_parameter(
            "a16", [P, (NFILL - xf8) * FILLW], bf16, isOutput=False)
    c8_ext = nc.declare_dram_parameter("c8", [P, NFILL * FILLW], fp8,
                                       isOutput=False)
    dyb_ext = nc.declare_dram_parameter("dyb", [P, NFILL * 3 * F], bf16,
                                        isOutput=False)
    # wb[:, 64*t : 64*(t+1)] = W_beta_t for t < 9; wb[:, 576:640] = W1
    wb_ext = nc.declare_dram_parameter("wb", [P, (NSLAB + 1) * NG], bf16,
                                       isOutput=False)
    out_ext = nc.declare_dram_parameter("out", [P, 2 * (NFILL + 1)], f32,
                                        isOutput=True)

    with tile.TileContext(nc) as tc:
        with tc.tile_pool(name="consts", bufs=1) as consts, \
             tc.tile_pool(name="a8io", bufs=2) as a8io, \
             tc.tile_pool(name="aio", bufs=2) as aio, \
             tc.tile_pool(name="ahio", bufs=2) as ahio, \
             tc.tile_pool(name="cio", bufs=3) as cio, \
             tc.tile_pool(name="dio", bufs=4) as dio, \
             tc.tile_pool(name="work", bufs=3) as work, \
             tc.tile_pool(name="aup", bufs=2) as aupp, \
             tc.tile_pool(name="epi", bufs=2) as epi, \
             tc.psum_pool(name="ps", bufs=4) as pspool:
            wb_t = consts.tile([P, (NSLAB + 1) * NG], bf16, name="wb",
                               tag="wb")
            nc.sync.dma_start(out=wb_t[:], in_=wb_ext[:])
            accs = consts.tile([P, 2 * (NFILL + 1)], f32, name="accs",
                               tag="accs")
            nc.vector.memset(accs[:], 0.0)

            def wview(t):
                return wb_t[:, NG * t:NG * (t + 1)]

            def dma_a(f):
                if f in f8set:
                    a_ch = a8io.tile([P, FILLW], fp8, tag="a8", name=f"a{f}")
                    fo = a8_slot[f]
                    src = a8_ext[:, fo * FILLW:(fo + 1) * FILLW]
                else:
                    a_ch = aio.tile([P, FILLW], bf16, tag="a16", name=f"a{f}")
                    fo = a16_slot[f]
                    src = a16_ext[:, fo * FILLW:(fo + 1) * FILLW]
                nc.sync.dma_start(out=a_ch[:], in_=src)
                return a_ch

            def dma_a_half(f, h):
                fo = a16_slot[f]
                a_ch = ahio.tile([P, HFILL], bf16, tag="a16h",
                                 name=f"a{f}h{h}")
                lo = fo * FILLW + h * HFILL
                nc.sync.dma_start(out=a_ch[:], in_=a16_ext[:, lo:lo + HFILL])
                return a_ch

            def dma_c(f):
                c_ch = cio.tile([P, FILLW], fp8, tag="c", name=f"c{f}")
                nc.sync.dma_start(out=c_ch[:],
                                  in_=c8_ext[:, f * FILLW:(f + 1) * FILLW])
                return c_ch

            def dma_dyb(f):
                dyb_ch = dio.tile([P, 3 * F], bf16, tag="dyb", name=f"dyb{f}")
                nc.sync.dma_start(
                    out=dyb_ch[:], in_=dyb_ext[:, f * 3 * F:(f + 1) * 3 * F])
                return dyb_ch

            def epilogue(slot, ps, dyb_ch, p0, np_, sq_dve=False):
                """Square-accumulate (psum+d-y) and (psum+d-b) for psum
                partitions [p0, p0+np_) into accs column pair `slot`.
                sq_dve: run the squares on DVE (tensor_tensor_reduce)
                instead of ACT, to balance engine load."""
                sl = slice(p0, p0 + np_)
                pv = ps[sl, :]
                d_v = dyb_ch[sl, 0:F]
                y_v = dyb_ch[sl, F:2 * F]
                b_v = dyb_ch[sl, 2 * F:3 * F]
                e0 = epi.tile([P, F], bf16, tag="e0", name=f"e0_{slot}")
                nc.vector.tensor_tensor(e0[sl, :], pv, d_v, add)
                e1 = epi.tile([P, F], bf16, tag="e1", name=f"e1_{slot}")
                nc.vector.tensor_tensor(e1[sl, :], e0[sl, :], y_v, sub)
                e2 = epi.tile([P, F], bf16, tag="e2", name=f"e2_{slot}")
                nc.vector.tensor_tensor(e2[sl, :], e0[sl, :], b_v, sub)
                s1 = epi.tile([P, F], bf16, tag="s1", name=f"s1_{slot}")
                s2 = epi.tile([P, F], bf16, tag="s2", name=f"s2_{slot}")
                a1 = accs[sl, 2 * slot:2 * slot + 1]
                a2 = accs[sl, 2 * slot + 1:2 * slot + 2]
                if sq_dve:
                    nc.vector.tensor_tensor_reduce(
                        s1[sl, :], e1[sl, :], e1[sl, :], 1.0, 0.0,
                        mult, add, a1)
                    nc.vector.tensor_tensor_reduce(
                        s2[sl, :], e2[sl, :], e2[sl, :], 1.0, 0.0,
                        mult, add, a2)
                else:
                    nc.scalar.activation(s1[sl, :], e1[sl, :], SQ,
                                         accum_out=a1)
                    nc.scalar.activation(s2[sl, :], e2[sl, :], SQ,
                                         accum_out=a2)

            def matmuls(out_v, a_view, prod_view):
                # beta matmuls first: they only need the a stream, so PE
                # starts while DVE is still computing prod
                for t in range(NSLAB):
                    nc.tensor.matmul(out_v, wview(t),
                                     a_view[:, t * F:(t + 1) * F],
                                     start=(t == 0), stop=False)
                for t in range(NSLAB):
                    nc.tensor.matmul(out_v, wview(NSLAB),
                                     prod_view[:, t * F:(t + 1) * F],
                                     start=False, stop=(t == NSLAB - 1))

            def half_work(f, b, a_v, a_is_fp8, c_v, ps_v):
                """One chunk: upcasts, multiply, 18 matmuls into ps_v.
                An fp8 a-stream is upcast on DVE for the multiply (copy at
                2 elem/cyc + 2x multiply beats a 1x mixed-dtype multiply in
                practice); PE reads the fp8 directly."""
                if a_is_fp8:
                    aup = aupp.tile([P, HFILL], bf16, tag="aup",
                                    name=f"au{f}_{b}")
                    nc.vector.tensor_copy(aup[:], a_v)
                    a_mul = aup[:]
                else:
                    a_mul = a_v
                bc = work.tile([P, HFILL], bf16, tag="bc", name=f"bc{f}_{b}")
                nc.scalar.activation(bc[:], c_v, COPY)
                prod = work.tile([P, HFILL], bf16, tag="prod",
                                 name=f"p{f}_{b}")
                nc.vector.tensor_tensor(prod[:], a_mul, bc[:], mult)
                matmuls(ps_v, a_v, prod[:])

            def fill_work(f, a_ch, c_ch):
                ps = pspool.tile([P, F], f32, tag="ps", name=f"ps{f}")
                for b in range(NBLK):
                    base = b * HFILL
                    half_work(f, b, a_ch[:, base:base + HFILL], f in f8set,
                              c_ch[:, base:base + HFILL],
                              ps[NG * b:NG * (b + 1), :])
                return ps

            def body():
                n_main = NFILL - 1 if taper else NFILL
                pend = []  # software pipeline: epilogue(f) after work(f+1)
                c_last = dyb_last = None
                for f in range(n_main):
                    a_ch = dma_a(f)
                    c_ch = dma_c(f)
                    dyb_ch = dma_dyb(f)
                    if f == n_main - 2 and taper:
                        # prefetch the taper fill's small streams so its
                        # a-halves are the only post-prefetch DMAs
                        c_last = dma_c(NFILL - 1)
                        dyb_last = dma_dyb(NFILL - 1)
                    if probe == "dma":
                        continue
                    ps = fill_work(f, a_ch, c_ch)
                    pend.append((f, ps, dyb_ch))
                    if len(pend) > 1:
                        fo, pso, dybo = pend.pop(0)
                        epilogue(fo, pso, dybo, 0, P, fo in sq_dve_set)
                if taper:
                    fl = NFILL - 1
                    if probe == "dma":
                        for h in range(NBLK):
                            dma_a_half(fl, h)
                    else:
                        psl = pspool.tile([P, F], f32, tag="ps", name="psl")
                        for h in range(NBLK):
                            a_h = dma_a_half(fl, h)
                            half_work(fl, h, a_h[:], False,
                                      c_last[:, h * HFILL:(h + 1) * HFILL],
                                      psl[NG * h:NG * (h + 1), :])
                            if pend:
                                fo, pso, dybo = pend.pop(0)
                                epilogue(fo, pso, dybo, 0, P,
                                         fo in sq_dve_set)
                            epilogue(fl + h, psl, dyb_last, NG * h, NG)
                for args in pend:
                    f, pso, dybo = args
                    epilogue(f, pso, dybo, 0, P, f in sq_dve_set)

            if hw_rep:
                with tc.For_i(0, hw_rep) as _:
                    body()
            else:
                body()

            nc.sync.dma_start(out=out_ext[:], in_=accs[:])
    nc.finalize()
    _cached[key] = nc
    return nc


def _prepare_in_maps(model_output, y, A, B_tl, beta_TL, xf8=XF8):
    model_output = np.asarray(model_output, dtype=np.float32)
    y = np.asarray(y, dtype=np.float32)
    A = np.asarray(A, dtype=np.float32)
    B_tl = np.asarray(B_tl, dtype=np.float32)
    beta_TL = np.asarray(beta_TL, dtype=np.float32)

    # wb: 9 beta-weight matrices then the ones matrix, each [128, 64]
    wb = np.zeros((P, (NSLAB + 1) * NG), dtype=np.float32)
    g_idx = np.arange(NG)
    for t in range(NSLAB):
        for s in range(NSLOT):
            wb[NSLOT * g_idx + s, NG * t + g_idx] = beta_TL[NSLAB * s + t]
    for s in range(NSLOT):
        wb[NSLOT * g_idx + s, NG * NSLAB + g_idx] = 1.0
    wb = wb.astype(ml_dtypes.bfloat16)

    def tcoef(arr):
        # [R, 18] -> partition 2g+s, free (chunk, slab t, col n), coef 9s+t
        return np.ascontiguousarray(
            arr.reshape(NG, NCHUNK, F, NSLOT, NSLAB)
            .transpose(0, 3, 1, 4, 2).reshape(P, NFILL * FILLW))

    def tfat(arr):
        # [R] -> [NFILL, 128(=64b+g), F]
        return arr.reshape(NG, NFILL, NBLK, F).transpose(1, 2, 0, 3) \
                  .reshape(NFILL, P, F)

    in_maps = []
    for i in range(N_CORES):
        lo, hi = i * R, min((i + 1) * R, N_TOTAL)
        mo_sh = np.zeros((R, NCOEF + 1), dtype=np.float32)
        mo_sh[:hi - lo] = model_output[lo:hi]
        a_sh = np.zeros((R, NCOEF), dtype=np.float32)
        a_sh[:hi - lo] = A[lo:hi]
        d_sh = mo_sh[:, NCOEF].copy()
        y_sh = np.zeros((R,), dtype=np.float32)
        y_sh[:hi - lo] = y[lo:hi, 0]
        b_sh = np.zeros((R,), dtype=np.float32)
        b_sh[:hi - lo] = B_tl[lo:hi, 0]

        a_t = tcoef(a_sh)
        c_t = tcoef(mo_sh[:, :NCOEF]).astype(ml_dtypes.float8_e4m3)
        dyb = np.ascontiguousarray(
            np.concatenate([tfat(d_sh), tfat(y_sh), tfat(b_sh)], axis=2)
            .transpose(1, 0, 2)
            .reshape(P, NFILL * 3 * F)).astype(ml_dtypes.bfloat16)
        m = {"c8": c_t, "dyb": dyb, "wb": wb}
        f8 = fp8_fills(xf8)
        f16 = [f for f in range(NFILL) if f not in f8]
        af = a_t.reshape(P, NFILL, FILLW)
        if f8:
            m["a8"] = np.ascontiguousarray(
                af[:, f8, :].reshape(P, -1)).astype(ml_dtypes.float8_e4m3)
        if f16:
            m["a16"] = np.ascontiguousarray(
                af[:, f16, :].reshape(P, -1)).astype(ml_dtypes.bfloat16)
        in_maps.append(m)
    return in_maps


def kernel(model_output, y, A, B_tl, beta_TL):
    nc = _build()
    in_maps = _prepare_in_maps(model_output, y, A, B_tl, beta_TL)
    res = run_bass_kernel_spmd(nc, in_maps, list(range(N_CORES)))
    total = 0.0
    for r in res.results:
        total += float(r["out"].astype(np.float64).sum())
    return np.asarray(total / N_TOTAL, dtype=np.float32)


# revision 35
# speedup vs baseline: 1.0474x; 1.0099x over previous
"""Trainium2 Bass kernel for the Tolles-Lawson custom loss.

reference:
    c = model_output[:, :18]; d = model_output[:, 18:19]
    tmp = sum(A * (beta_TL + c), axis=1, keepdims=True) + d
    L = mean((tmp - y)^2) + mean((tmp - B_tl)^2)

Sharding: pure data parallel over rows on 8 cores (R = 524,288 rows per
core, tail zero-padded; zero rows contribute 0 to both sums). Per-core
partial sums land in accs [128, 18]; the all-reduce is host-side.

Layout (per core), "transposed": rows split into 64 groups of NCG=8192;
SBUF partition p = 2*g + s holds slot s (coefficients 9s..9s+8) of
group g, rows along the free axis as 9 slabs of 512 columns per chunk.
The 18-way row-reduction runs on the TensorEngine: for each 512-column
chunk, 9 accumulating matmuls (one per coefficient slab t) with a
block-ones stationary W1 [128, 64] (W1[2g+s, g] = 1) compute
    psum[64b+g, n] = sum_s sum_t prod[2g+s, (b,t,n)]
(b = chunk parity selecting the PSUM base partition 0/64 - matmul
output base must be 0/32/64). The beta term sum_j A_ij * beta_j is
folded into 9 more matmuls with W_bt[2g+s, g] = beta[9s+t] applied to
the raw A stream, so no engine ever materializes (c + beta).
Two chunks fill a fat [128, 512] PSUM tile; the epilogue runs at full
partition width.

Engine split per fill [128, 9216] (= 65,536 rows):
  ACT : fp8->bf16 upcast of c (Copy), squares+accum of e1/e2
  DVE : fp8->bf16 upcast of a (first XF8 fills), prod = a * c (2x mode),
        e0 = psum + d, e1/e2 = e0 - y/b
  PE  : 2 blocks x (9 beta-matmuls on a + 9 ones-matmuls on prod)
  DMA : a [128,9216] (fp8 first XF8 fills, bf16 after), c fp8(e4m3),
        dyb [128,1536] bf16

Precision: c is always fp8 e4m3, a is fp8 on XF8=4 of 8 fills,
interleaved with bf16-a fills so per-fill engine load stays under the
DMA cadence (~26.8 MB/core/pass vs 39 MB bf16 baseline). The
per-element quantization noise is random and washes out in the 4M-row
mean (measured rel err 1.1e-3 vs the 2e-2 budget).

Drain taper: each measured pass (and any single launch) pays the
pipeline drain after the last DMA. The last fill is bf16 (no upcast in
its chain), its c/dyb are prefetched early, and it is processed as two
half-fills of one 512-column chunk each so the post-DMA tail is a short
[128,4608] multiply + 18 matmuls + [64,512] epilogue.

Measured (For_i slope, 8 cores): 102.9 us/pass vs 151.8 us for the
previous row-major DVE-tree kernel and 96.9->81.1 us for the DMA
streams alone (i.e. ~326-330 GB/s/core effective, 91% of the 358 GB/s
HBM-per-core limit; the kernel runs ~22 us above its DMA floor, the
residual being For_i's per-iteration all-engine barrier drain plus
cross-engine dependency stalls). Engine busy estimates per pass:
DMA 81 us, ACT ~79 us (upcasts + squares), PE ~77 us (288 self-loading
matmuls), DVE ~72 us (upcasts, multiplies, epilogue).
"""

import numpy as np
import ml_dtypes

import concourse.bacc as bacc
import concourse.mybir as mybir
from concourse import tile
from concourse.bass_utils import run_bass_kernel_spmd

N_TOTAL = 4_000_000
NCOEF = 18
NG = 64                # row groups per core
NSLOT = 2              # coefficient slots per group
NSLAB = 9              # coefficients per slot
P = NG * NSLOT         # 128 SBUF partitions
F = 512                # chunk columns (one PSUM bank of f32)
NBLK = 2               # chunks per fill -> NBLK*NG = 128 psum partitions
HFILL = F * NSLAB      # 4608 free elements per half-fill (one chunk)
FILLW = HFILL * NBLK   # 9216 free elements per fill per partition
NFILL = 8              # fills per core per pass
NCHUNK = NBLK * NFILL  # 16 chunks per group
NCG = F * NCHUNK       # 8192 columns per group
R = NG * NCG           # 524288 rows per core
N_CORES = 8

XF8 = 4                # number of fills whose a-stream is fp8
TAPER = True           # process the last fill as two half-fills
SQMOVE = 0             # bf16-a fills whose squares run on DVE, not ACT
                       # (tensor_tensor_reduce trips a walrus codegen error
                       # on this stack; keep 0)


def fp8_fills(xf8):
    """Spread the fp8-a fills among fills 0..NFILL-2 (the taper fill is
    always bf16) so heavy upcast fills alternate with light ones."""
    if xf8 <= 0:
        return []
    return sorted(set(
        int(round(i * (NFILL - 2) / max(xf8 - 1, 1))) for i in range(xf8)))

f32 = mybir.dt.float32
bf16 = mybir.dt.bfloat16
fp8 = mybir.dt.float8e4
add = mybir.AluOpType.add
sub = mybir.AluOpType.subtract
mult = mybir.AluOpType.mult
COPY = mybir.ActivationFunctionType.Copy
SQ = mybir.ActivationFunctionType.Square

_cached = {}


def _build(hw_rep=0, dma_only=False, probe=None, xf8=XF8, taper=TAPER,
           sqmove=SQMOVE):
    """hw_rep > 0 wraps the pass in a For_i hardware loop (bench only).

    probe: None = full kernel; "dma" = DMAs only.
    """
    if dma_only:
        probe = "dma"
    key = (hw_rep, probe, xf8, taper, sqmove)
    if key in _cached:
        return _cached[key]
    f8set = set(fp8_fills(xf8))
    sq_dve_set = set(
        [f for f in range(NFILL - 1) if f not in f8set][:sqmove])
    # halves whose c-upcast runs on DVE instead of ACT (load balance:
    # ACT is the busiest engine; DVE has a little slack on bf16-a fills)
    up_dve_set = set(
        (f, 0) for f in [f for f in range(NFILL - 1) if f not in f8set][:2])
    a8_slot = {f: i for i, f in enumerate(sorted(f8set))}
    a16_slot = {f: i for i, f in
                enumerate(f for f in range(NFILL) if f not in f8set)}

    nc = bacc.Bacc(None)
    a8_ext = c8_ext = a16_ext = None
    if xf8 > 0:
        a8_ext = nc.declare_dram_parameter("a8", [P, xf8 * FILLW], fp8,
                                           isOutput=False)
    if xf8 < NFILL:
        a16_ext = nc.declare_dram_parameter(
            "a16", [P, (NFILL - xf8) * FILLW], bf16, isOutput=False)
    c8_ext = nc.declare_dram_parameter("c8", [P, NFILL * FILLW], fp8,
                                       isOutput=False)
    dyb_ext = nc.declare_dram_parameter("dyb", [P, NFILL * 3 * F], bf16,
                                        isOutput=False)
    # wb[:, 64*t : 64*(t+1)] = W_beta_t for t < 9; wb[:, 576:640] = W1
    wb_ext = nc.declare_dram_parameter("wb", [P, (NSLAB + 1) * NG], bf16,
                                       isOutput=False)
    out_ext = nc.declare_dram_parameter("out", [P, 2 * (NFILL + 1)], f32,
                                        isOutput=True)

    with tile.TileContext(nc) as tc:
        with tc.tile_pool(name="consts", bufs=1) as consts, \
             tc.tile_pool(name="a8io", bufs=2) as a8io, \
             tc.tile_pool(name="aio", bufs=2) as aio, \
             tc.tile_pool(name="ahio", bufs=2) as ahio, \
             tc.tile_pool(name="cio", bufs=3) as cio, \
             tc.tile_pool(name="dio", bufs=4) as dio, \
             tc.tile_pool(name="work", bufs=3) as work, \
             tc.tile_pool(name="aup", bufs=2) as aupp, \
             tc.tile_pool(name="epi", bufs=2) as epi, \
             tc.psum_pool(name="ps", bufs=4) as pspool:
            wb_t = consts.tile([P, (NSLAB + 1) * NG], bf16, name="wb",
                               tag="wb")
            nc.sync.dma_start(out=wb_t[:], in_=wb_ext[:])
            accs = consts.tile([P, 2 * (NFILL + 1)], f32, name="accs",
                               tag="accs")
            nc.vector.memset(accs[:], 0.0)

            def wview(t):
                return wb_t[:, NG * t:NG * (t + 1)]

            def dma_a(f):
                if f in f8set:
                    a_ch = a8io.tile([P, FILLW], fp8, tag="a8", name=f"a{f}")
                    fo = a8_slot[f]
                    src = a8_ext[:, fo * FILLW:(fo + 1) * FILLW]
                else:
                    a_ch = aio.tile([P, FILLW], bf16, tag="a16", name=f"a{f}")
                    fo = a16_slot[f]
                    src = a16_ext[:, fo * FILLW:(fo + 1) * FILLW]
                nc.sync.dma_start(out=a_ch[:], in_=src)
                return a_ch

            def dma_a_half(f, h):
                fo = a16_slot[f]
                a_ch = ahio.tile([P, HFILL], bf16, tag="a16h",
                                 name=f"a{f}h{h}")
                lo = fo * FILLW + h * HFILL
                nc.sync.dma_start(out=a_ch[:], in_=a16_ext[:, lo:lo + HFILL])
                return a_ch

            def dma_c(f):
                c_ch = cio.tile([P, FILLW], fp8, tag="c", name=f"c{f}")
                nc.sync.dma_start(out=c_ch[:],
                                  in_=c8_ext[:, f * FILLW:(f + 1) * FILLW])
                return c_ch

            def dma_dyb(f):
                dyb_ch = dio.tile([P, 3 * F], bf16, tag="dyb", name=f"dyb{f}")
                nc.sync.dma_start(
                    out=dyb_ch[:], in_=dyb_ext[:, f * 3 * F:(f + 1) * 3 * F])
                return dyb_ch

            def epilogue(slot, ps, dyb_ch, p0, np_, sq_dve=False):
                """Square-accumulate (psum+d-y) and (psum+d-b) for psum
                partitions [p0, p0+np_) into accs column pair `slot`.
                sq_dve: run the squares on DVE (tensor_tensor_reduce)
                instead of ACT, to balance engine load."""
                sl = slice(p0, p0 + np_)
                pv = ps[sl, :]
                d_v = dyb_ch[sl, 0:F]
                y_v = dyb_ch[sl, F:2 * F]
                b_v = dyb_ch[sl, 2 * F:3 * F]
                e0 = epi.tile([P, F], bf16, tag="e0", name=f"e0_{slot}")
                nc.vector.tensor_tensor(e0[sl, :], pv, d_v, add)
                e1 = epi.tile([P, F], bf16, tag="e1", name=f"e1_{slot}")
                nc.vector.tensor_tensor(e1[sl, :], e0[sl, :], y_v, sub)
                e2 = epi.tile([P, F], bf16, tag="e2", name=f"e2_{slot}")
                nc.vector.tensor_tensor(e2[sl, :], e0[sl, :], b_v, sub)
                s1 = epi.tile([P, F], bf16, tag="s1", name=f"s1_{slot}")
                s2 = epi.tile([P, F], bf16, tag="s2", name=f"s2_{slot}")
                a1 = accs[sl, 2 * slot:2 * slot + 1]
                a2 = accs[sl, 2 * slot + 1:2 * slot + 2]
                if sq_dve:
                    nc.vector.tensor_tensor_reduce(
                        s1[sl, :], e1[sl, :], e1[sl, :], 1.0, 0.0,
                        mult, add, a1)
                    nc.vector.tensor_tensor_reduce(
                        s2[sl, :], e2[sl, :], e2[sl, :], 1.0, 0.0,
                        mult, add, a2)
                else:
                    nc.scalar.activation(s1[sl, :], e1[sl, :], SQ,
                                         accum_out=a1)
                    nc.scalar.activation(s2[sl, :], e2[sl, :], SQ,
                                         accum_out=a2)

            def matmuls(out_v, a_view, prod_view):
                # beta matmuls first: they only need the a stream, so PE
                # starts while DVE is still computing prod
                for t in range(NSLAB):
                    nc.tensor.matmul(out_v, wview(t),
                                     a_view[:, t * F:(t + 1) * F],
                                     start=(t == 0), stop=False)
                for t in range(NSLAB):
                    nc.tensor.matmul(out_v, wview(NSLAB),
                                     prod_view[:, t * F:(t + 1) * F],
                                     start=False, stop=(t == NSLAB - 1))

            def half_work(f, b, a_v, a_is_fp8, c_v, ps_v):
                """One chunk: upcasts, multiply, 18 matmuls into ps_v.
                An fp8 a-stream is upcast on DVE for the multiply (copy at
                2 elem/cyc + 2x multiply beats a 1x mixed-dtype multiply in
                practice); PE reads the fp8 directly."""
                if a_is_fp8:
                    aup = aupp.tile([P, HFILL], bf16, tag="aup",
                                    name=f"au{f}_{b}")
                    nc.vector.tensor_copy(aup[:], a_v)
                    a_mul = aup[:]
                else:
                    a_mul = a_v
                bc = work.tile([P, HFILL], bf16, tag="bc", name=f"bc{f}_{b}")
                if (f, b) in up_dve_set:
                    nc.vector.tensor_copy(bc[:], c_v)
                else:
                    nc.scalar.activation(bc[:], c_v, COPY)
                prod = work.tile([P, HFILL], bf16, tag="prod",
                                 name=f"p{f}_{b}")
                nc.vector.tensor_tensor(prod[:], a_mul, bc[:], mult)
                matmuls(ps_v, a_v, prod[:])

            def fill_work(f, a_ch, c_ch):
                ps = pspool.tile([P, F], f32, tag="ps", name=f"ps{f}")
                for b in range(NBLK):
                    base = b * HFILL
                    half_work(f, b, a_ch[:, base:base + HFILL], f in f8set,
                              c_ch[:, base:base + HFILL],
                              ps[NG * b:NG * (b + 1), :])
                return ps

            def body():
                n_main = NFILL - 1 if taper else NFILL
                pend = []  # software pipeline: epilogue(f) after work(f+1)
                c_last = dyb_last = None
                for f in range(n_main):
                    # c first: the upcast chain is the longest consumer
                    c_ch = dma_c(f)
                    a_ch = dma_a(f)
                    dyb_ch = dma_dyb(f)
                    if f == n_main - 2 and taper:
                        # prefetch the taper fill's small streams so its
                        # a-halves are the only post-prefetch DMAs
                        c_last = dma_c(NFILL - 1)
                        dyb_last = dma_dyb(NFILL - 1)
                    if probe == "dma":
                        continue
                    ps = fill_work(f, a_ch, c_ch)
                    pend.append((f, ps, dyb_ch))
                    if len(pend) > 1:
                        fo, pso, dybo = pend.pop(0)
                        epilogue(fo, pso, dybo, 0, P, fo in sq_dve_set)
                if taper:
                    fl = NFILL - 1
                    if probe == "dma":
                        for h in range(NBLK):
                            dma_a_half(fl, h)
                    else:
                        psl = pspool.tile([P, F], f32, tag="ps", name="psl")
                        for h in range(NBLK):
                            a_h = dma_a_half(fl, h)
                            half_work(fl, h, a_h[:], False,
                                      c_last[:, h * HFILL:(h + 1) * HFILL],
                                      psl[NG * h:NG * (h + 1), :])
                            if pend:
                                fo, pso, dybo = pend.pop(0)
                                epilogue(fo, pso, dybo, 0, P,
                                         fo in sq_dve_set)
                            epilogue(fl + h, psl, dyb_last, NG * h, NG)
                for args in pend:
                    f, pso, dybo = args
                    epilogue(f, pso, dybo, 0, P, f in sq_dve_set)

            if hw_rep:
                with tc.For_i(0, hw_rep) as _:
                    body()
            else:
                body()

            nc.sync.dma_start(out=out_ext[:], in_=accs[:])
    nc.finalize()
    _cached[key] = nc
    return nc


def _prepare_in_maps(model_output, y, A, B_tl, beta_TL, xf8=XF8):
    model_output = np.asarray(model_output, dtype=np.float32)
    y = np.asarray(y, dtype=np.float32)
    A = np.asarray(A, dtype=np.float32)
    B_tl = np.asarray(B_tl, dtype=np.float32)
    beta_TL = np.asarray(beta_TL, dtype=np.float32)

    # wb: 9 beta-weight matrices then the ones matrix, each [128, 64]
    wb = np.zeros((P, (NSLAB + 1) * NG), dtype=np.float32)
    g_idx = np.arange(NG)
    for t in range(NSLAB):
        for s in range(NSLOT):
            wb[NSLOT * g_idx + s, NG * t + g_idx] = beta_TL[NSLAB * s + t]
    for s in range(NSLOT):
        wb[NSLOT * g_idx + s, NG * NSLAB + g_idx] = 1.0
    wb = wb.astype(ml_dtypes.bfloat16)

    def tcoef(arr):
        # [R, 18] -> partition 2g+s, free (chunk, slab t, col n), coef 9s+t
        return np.ascontiguousarray(
            arr.reshape(NG, NCHUNK, F, NSLOT, NSLAB)
            .transpose(0, 3, 1, 4, 2).reshape(P, NFILL * FILLW))

    def tfat(arr):
        # [R] -> [NFILL, 128(=64b+g), F]
        return arr.reshape(NG, NFILL, NBLK, F).transpose(1, 2, 0, 3) \
                  .reshape(NFILL, P, F)

    in_maps = []
    for i in range(N_CORES):
        lo, hi = i * R, min((i + 1) * R, N_TOTAL)
        mo_sh = np.zeros((R, NCOEF + 1), dtype=np.float32)
        mo_sh[:hi - lo] = model_output[lo:hi]
        a_sh = np.zeros((R, NCOEF), dtype=np.float32)
        a_sh[:hi - lo] = A[lo:hi]
        d_sh = mo_sh[:, NCOEF].copy()
        y_sh = np.zeros((R,), dtype=np.float32)
        y_sh[:hi - lo] = y[lo:hi, 0]
        b_sh = np.zeros((R,), dtype=np.float32)
        b_sh[:hi - lo] = B_tl[lo:hi, 0]

        a_t = tcoef(a_sh)
        c_t = tcoef(mo_sh[:, :NCOEF]).astype(ml_dtypes.float8_e4m3)
        dyb = np.ascontiguousarray(
            np.concatenate([tfat(d_sh), tfat(y_sh), tfat(b_sh)], axis=2)
            .transpose(1, 0, 2)
            .reshape(P, NFILL * 3 * F)).astype(ml_dtypes.bfloat16)
        m = {"c8": c_t, "dyb": dyb, "wb": wb}
        f8 = fp8_fills(xf8)
        f16 = [f for f in range(NFILL) if f not in f8]
        af = a_t.reshape(P, NFILL, FILLW)
        if f8:
            m["a8"] = np.ascontiguousarray(
                af[:, f8, :].reshape(P, -1)).astype(ml_dtypes.float8_e4m3)
        if f16:
            m["a16"] = np.ascontiguousarray(
                af[:, f16, :].reshape(P, -1)).astype(ml_dtypes.bfloat16)
        in_maps.append(m)
    return in_maps


def kernel(model_output, y, A, B_tl, beta_TL):
    nc = _build()
    in_maps = _prepare_in_maps(model_output, y, A, B_tl, beta_TL)
    res = run_bass_kernel_spmd(nc, in_maps, list(range(N_CORES)))
    total = 0.0
    for r in res.results:
        total += float(r["out"].astype(np.float64).sum())
    return np.asarray(total / N_TOTAL, dtype=np.float32)


# revision 38
# speedup vs baseline: 1.1682x; 1.1153x over previous
"""Trainium2 Bass kernel for the Tolles-Lawson custom loss.

reference:
    c = model_output[:, :18]; d = model_output[:, 18:19]
    tmp = sum(A * (beta_TL + c), axis=1, keepdims=True) + d
    L = mean((tmp - y)^2) + mean((tmp - B_tl)^2)

Sharding: pure data parallel over rows on 8 cores (R = 524,288 rows per
core, tail zero-padded; zero rows contribute 0 to both sums). Per-core
partial sums land in accs [128, 18]; the all-reduce is host-side.

Layout (per core), "transposed": rows split into 64 groups of NCG=8192;
SBUF partition p = 2*g + s holds slot s (coefficients 9s..9s+8) of
group g, rows along the free axis as 9 slabs of 512 columns per chunk.
The 18-way row-reduction runs on the TensorEngine: for each 512-column
chunk, 9 accumulating matmuls (one per coefficient slab t) with a
block-ones stationary W1 [128, 64] (W1[2g+s, g] = 1) compute
    psum[64b+g, n] = sum_s sum_t prod[2g+s, (b,t,n)]
(b = chunk parity selecting the PSUM base partition 0/64 - matmul
output base must be 0/32/64). The beta term sum_j A_ij * beta_j is
folded into 9 more matmuls with W_bt[2g+s, g] = beta[9s+t] applied to
the raw A stream, so no engine ever materializes (c + beta).
Two chunks fill a fat [128, 512] PSUM tile; the epilogue runs at full
partition width.

Engine split per fill [128, 9216] (= 65,536 rows):
  ACT : fp8->bf16 upcast of c (Copy), squares+accum of e1/e2
  DVE : fp8->bf16 upcast of a (first XF8 fills), prod = a * c (2x mode),
        e0 = psum + d, e1/e2 = e0 - y/b
  PE  : 2 blocks x (9 beta-matmuls on a + 9 ones-matmuls on prod)
  DMA : a [128,9216] (fp8 first XF8 fills, bf16 after), c fp8(e4m3),
        dyb [128,1536] bf16

Precision: c is always fp8 e4m3, a is fp8 on XF8=4 of 8 fills,
interleaved with bf16-a fills so per-fill engine load stays under the
DMA cadence (~26.8 MB/core/pass vs 39 MB bf16 baseline). The
per-element quantization noise is random and washes out in the 4M-row
mean (measured rel err 1.1e-3 vs the 2e-2 budget).

Drain taper: each measured pass (and any single launch) pays the
pipeline drain after the last DMA. The last fill is bf16 (no upcast in
its chain), its c/dyb are prefetched early, and it is processed as two
half-fills of one 512-column chunk each so the post-DMA tail is a short
[128,4608] multiply + 18 matmuls + [64,512] epilogue.

Measured (For_i slope, 8 cores, run-to-run ~+-3 us): ~103 us/pass vs
151.8 us for the previous row-major DVE-tree kernel, and 81.1 us for
the DMA streams alone (~330 GB/s/core effective, 92% of the 358 GB/s
HBM-per-core limit; the kernel runs ~20 us above its DMA floor, the
residual being For_i's per-iteration all-engine barrier drain plus
cross-engine dependency stalls). Engine busy estimates per pass:
DMA 81 us, ACT ~75 us (upcasts + squares), PE ~77 us (288 self-loading
matmuls), DVE ~75 us (upcasts, multiplies, epilogue).
"""

import numpy as np
import ml_dtypes

import concourse.bacc as bacc
import concourse.mybir as mybir
from concourse import tile
from concourse.bass_utils import run_bass_kernel_spmd

N_TOTAL = 4_000_000
NCOEF = 18
NG = 64                # row groups per core
NSLOT = 2              # coefficient slots per group
NSLAB = 9              # coefficients per slot
P = NG * NSLOT         # 128 SBUF partitions
F = 512                # chunk columns (one PSUM bank of f32)
NBLK = 2               # chunks per fill -> NBLK*NG = 128 psum partitions
HFILL = F * NSLAB      # 4608 free elements per half-fill (one chunk)
FILLW = HFILL * NBLK   # 9216 free elements per fill per partition
NFILL = 8              # fills per core per pass
NCHUNK = NBLK * NFILL  # 16 chunks per group
NCG = F * NCHUNK       # 8192 columns per group
R = NG * NCG           # 524288 rows per core
N_CORES = 8

XF8 = 4                # number of fills whose a-stream is fp8
TAPER = True           # process the last fill as two half-fills
SQMOVE = 0             # bf16-a fills whose squares run on DVE, not ACT
                       # (tensor_tensor_reduce trips a walrus codegen error
                       # on this stack; keep 0)


def fp8_fills(xf8):
    """Spread the fp8-a fills among fills 0..NFILL-2 (the taper fill is
    always bf16) so heavy upcast fills alternate with light ones."""
    if xf8 <= 0:
        return []
    return sorted(set(
        int(round(i * (NFILL - 2) / max(xf8 - 1, 1))) for i in range(xf8)))

f32 = mybir.dt.float32
bf16 = mybir.dt.bfloat16
fp8 = mybir.dt.float8e4
add = mybir.AluOpType.add
sub = mybir.AluOpType.subtract
mult = mybir.AluOpType.mult
COPY = mybir.ActivationFunctionType.Copy
SQ = mybir.ActivationFunctionType.Square

_cached = {}


def _build(hw_rep=0, dma_only=False, probe=None, xf8=XF8, taper=TAPER,
           sqmove=SQMOVE):
    """hw_rep > 0 wraps the pass in a For_i hardware loop (bench only).

    probe: None = full kernel; "dma" = DMAs only.
    """
    if dma_only:
        probe = "dma"
    key = (hw_rep, probe, xf8, taper, sqmove)
    if key in _cached:
        return _cached[key]
    f8set = set(fp8_fills(xf8))
    sq_dve_set = set(
        [f for f in range(NFILL - 1) if f not in f8set][:sqmove])
    # halves whose c-upcast runs on DVE instead of ACT (load balance:
    # ACT is the busiest engine; DVE has a little slack on bf16-a fills)
    up_dve_set = set(
        (f, 0) for f in [f for f in range(NFILL - 1) if f not in f8set][:2])
    a8_slot = {f: i for i, f in enumerate(sorted(f8set))}
    a16_slot = {f: i for i, f in
                enumerate(f for f in range(NFILL) if f not in f8set)}

    nc = bacc.Bacc(None)
    a8_ext = c8_ext = a16_ext = None
    if xf8 > 0:
        a8_ext = nc.declare_dram_parameter("a8", [P, xf8 * FILLW], fp8,
                                           isOutput=False)
    if xf8 < NFILL:
        a16_ext = nc.declare_dram_parameter(
            "a16", [P, (NFILL - xf8) * FILLW], bf16, isOutput=False)
    c8_ext = nc.declare_dram_parameter("c8", [P, NFILL * FILLW], fp8,
                                       isOutput=False)
    dyb_ext = nc.declare_dram_parameter("dyb", [P, NFILL * 3 * F], bf16,
                                        isOutput=False)
    # wb[:, 64*t : 64*(t+1)] = W_beta_t for t < 9; wb[:, 576:640] = W1
    wb_ext = nc.declare_dram_parameter("wb", [P, (NSLAB + 1) * NG], bf16,
                                       isOutput=False)
    out_ext = nc.declare_dram_parameter("out", [P, 2 * (NFILL + 1)], f32,
                                        isOutput=True)

    with tile.TileContext(nc) as tc:
        with tc.tile_pool(name="consts", bufs=1) as consts, \
             tc.tile_pool(name="a8io", bufs=2) as a8io, \
             tc.tile_pool(name="aio", bufs=2) as aio, \
             tc.tile_pool(name="ahio", bufs=2) as ahio, \
             tc.tile_pool(name="cio", bufs=3) as cio, \
             tc.tile_pool(name="dio", bufs=4) as dio, \
             tc.tile_pool(name="work", bufs=3) as work, \
             tc.tile_pool(name="aup", bufs=2) as aupp, \
             tc.tile_pool(name="epi", bufs=2) as epi, \
             tc.psum_pool(name="ps", bufs=4) as pspool:
            wb_t = consts.tile([P, (NSLAB + 1) * NG], bf16, name="wb",
                               tag="wb")
            nc.sync.dma_start(out=wb_t[:], in_=wb_ext[:])
            accs = consts.tile([P, 2 * (NFILL + 1)], f32, name="accs",
                               tag="accs")
            nc.vector.memset(accs[:], 0.0)

            def wview(t):
                return wb_t[:, NG * t:NG * (t + 1)]

            def dma_a(f):
                if f in f8set:
                    a_ch = a8io.tile([P, FILLW], fp8, tag="a8", name=f"a{f}")
                    fo = a8_slot[f]
                    src = a8_ext[:, fo * FILLW:(fo + 1) * FILLW]
                else:
                    a_ch = aio.tile([P, FILLW], bf16, tag="a16", name=f"a{f}")
                    fo = a16_slot[f]
                    src = a16_ext[:, fo * FILLW:(fo + 1) * FILLW]
                nc.sync.dma_start(out=a_ch[:], in_=src)
                return a_ch

            def dma_a_half(f, h):
                fo = a16_slot[f]
                a_ch = ahio.tile([P, HFILL], bf16, tag="a16h",
                                 name=f"a{f}h{h}")
                lo = fo * FILLW + h * HFILL
                nc.sync.dma_start(out=a_ch[:], in_=a16_ext[:, lo:lo + HFILL])
                return a_ch

            def dma_c(f):
                c_ch = cio.tile([P, FILLW], fp8, tag="c", name=f"c{f}")
                nc.sync.dma_start(out=c_ch[:],
                                  in_=c8_ext[:, f * FILLW:(f + 1) * FILLW])
                return c_ch

            def dma_dyb(f):
                dyb_ch = dio.tile([P, 3 * F], bf16, tag="dyb", name=f"dyb{f}")
                nc.sync.dma_start(
                    out=dyb_ch[:], in_=dyb_ext[:, f * 3 * F:(f + 1) * 3 * F])
                return dyb_ch

            def epilogue(slot, ps, dyb_ch, p0, np_, sq_dve=False):
                """Square-accumulate (psum+d-y) and (psum+d-b) for psum
                partitions [p0, p0+np_) into accs column 2*slot. e1 and e2
                are packed into one [*, 1024] tile so a single ACT
                Square+accum covers both (accs column 2*slot+1 stays 0)."""
                sl = slice(p0, p0 + np_)
                pv = ps[sl, :]
                d_v = dyb_ch[sl, 0:F]
                y_v = dyb_ch[sl, F:2 * F]
                b_v = dyb_ch[sl, 2 * F:3 * F]
                e0 = epi.tile([P, F], bf16, tag="e0", name=f"e0_{slot}")
                nc.vector.tensor_tensor(e0[sl, :], pv, d_v, add)
                e12 = epi.tile([P, 2 * F], bf16, tag="e12",
                               name=f"e12_{slot}")
                nc.vector.tensor_tensor(e12[sl, 0:F], e0[sl, :], y_v, sub)
                nc.vector.tensor_tensor(e12[sl, F:2 * F], e0[sl, :], b_v,
                                        sub)
                s12 = epi.tile([P, 2 * F], bf16, tag="s12",
                               name=f"s12_{slot}")
                nc.scalar.activation(
                    s12[sl, :], e12[sl, :], SQ,
                    accum_out=accs[sl, 2 * slot:2 * slot + 1])

            def matmuls(out_v, a_view, prod_view):
                # beta matmuls first: they only need the a stream, so PE
                # starts while DVE is still computing prod
                for t in range(NSLAB):
                    nc.tensor.matmul(out_v, wview(t),
                                     a_view[:, t * F:(t + 1) * F],
                                     start=(t == 0), stop=False)
                for t in range(NSLAB):
                    nc.tensor.matmul(out_v, wview(NSLAB),
                                     prod_view[:, t * F:(t + 1) * F],
                                     start=False, stop=(t == NSLAB - 1))

            def half_work(f, b, a_v, a_is_fp8, c_v, ps_v):
                """One chunk: upcasts, multiply, 18 matmuls into ps_v.
                An fp8 a-stream is upcast on DVE for the multiply (copy at
                2 elem/cyc + 2x multiply beats a 1x mixed-dtype multiply in
                practice); PE reads the fp8 directly."""
                if a_is_fp8:
                    aup = aupp.tile([P, HFILL], bf16, tag="aup",
                                    name=f"au{f}_{b}")
                    nc.vector.tensor_copy(aup[:], a_v)
                    a_mul = aup[:]
                else:
                    a_mul = a_v
                bc = work.tile([P, HFILL], bf16, tag="bc", name=f"bc{f}_{b}")
                if (f, b) in up_dve_set:
                    nc.vector.tensor_copy(bc[:], c_v)
                else:
                    nc.scalar.activation(bc[:], c_v, COPY)
                prod = work.tile([P, HFILL], bf16, tag="prod",
                                 name=f"p{f}_{b}")
                nc.vector.tensor_tensor(prod[:], a_mul, bc[:], mult)
                matmuls(ps_v, a_v, prod[:])

            def fill_work(f, a_ch, c_ch):
                ps = pspool.tile([P, F], f32, tag="ps", name=f"ps{f}")
                for b in range(NBLK):
                    base = b * HFILL
                    half_work(f, b, a_ch[:, base:base + HFILL], f in f8set,
                              c_ch[:, base:base + HFILL],
                              ps[NG * b:NG * (b + 1), :])
                return ps

            def body():
                n_main = NFILL - 1 if taper else NFILL
                pend = []  # software pipeline: epilogue(f) after work(f+1)
                c_last = dyb_last = None
                for f in range(n_main):
                    # c first: the upcast chain is the longest consumer
                    c_ch = dma_c(f)
                    a_ch = dma_a(f)
                    dyb_ch = dma_dyb(f)
                    if f == n_main - 2 and taper:
                        # prefetch the taper fill's small streams so its
                        # a-halves are the only post-prefetch DMAs
                        c_last = dma_c(NFILL - 1)
                        dyb_last = dma_dyb(NFILL - 1)
                    if probe == "dma":
                        continue
                    ps = fill_work(f, a_ch, c_ch)
                    pend.append((f, ps, dyb_ch))
                    if len(pend) > 1:
                        fo, pso, dybo = pend.pop(0)
                        epilogue(fo, pso, dybo, 0, P, fo in sq_dve_set)
                if taper:
                    fl = NFILL - 1
                    if probe == "dma":
                        for h in range(NBLK):
                            dma_a_half(fl, h)
                    else:
                        psl = pspool.tile([P, F], f32, tag="ps", name="psl")
                        for h in range(NBLK):
                            # the c upcast only needs the prefetched c_last,
                            # so it is never on the post-DMA critical path
                            bch = work.tile([P, HFILL], bf16, tag="bc",
                                            name=f"bcl{h}")
                            nc.scalar.activation(
                                bch[:], c_last[:, h * HFILL:(h + 1) * HFILL],
                                COPY)
                            ps_v = psl[NG * h:NG * (h + 1), :]
                            # slab-split the a DMA (5+4 slabs) so the last
                            # transfer's dependent chain is half as long
                            for ta, tb in ((0, 5), (5, NSLAB)):
                                nel = (tb - ta) * F
                                a_q = ahio.tile([P, HFILL], bf16, tag="a16h",
                                                name=f"a{fl}h{h}q{ta}")
                                lo = (a16_slot[fl] * FILLW + h * HFILL
                                      + ta * F)
                                nc.sync.dma_start(
                                    out=a_q[:, :nel],
                                    in_=a16_ext[:, lo:lo + nel])
                                prod = work.tile([P, HFILL], bf16,
                                                 tag="prod",
                                                 name=f"pl{h}q{ta}")
                                nc.vector.tensor_tensor(
                                    prod[:, :nel], a_q[:, :nel],
                                    bch[:, ta * F:tb * F], mult)
                                for t in range(ta, tb):
                                    nc.tensor.matmul(
                                        ps_v, wview(t),
                                        a_q[:, (t - ta) * F:(t - ta + 1) * F],
                                        start=(t == 0), stop=False)
                                for t in range(ta, tb):
                                    nc.tensor.matmul(
                                        ps_v, wview(NSLAB),
                                        prod[:, (t - ta) * F:(t - ta + 1) * F],
                                        start=False,
                                        stop=(t == NSLAB - 1))
                            if pend:
                                fo, pso, dybo = pend.pop(0)
                                epilogue(fo, pso, dybo, 0, P,
                                         fo in sq_dve_set)
                            epilogue(fl + h, psl, dyb_last, NG * h, NG)
                for args in pend:
                    f, pso, dybo = args
                    epilogue(f, pso, dybo, 0, P, f in sq_dve_set)

            if hw_rep:
                with tc.For_i(0, hw_rep) as _:
                    body()
            else:
                body()

            nc.sync.dma_start(out=out_ext[:], in_=accs[:])
    nc.finalize()
    _cached[key] = nc
    return nc


def _prepare_in_maps(model_output, y, A, B_tl, beta_TL, xf8=XF8):
    model_output = np.asarray(model_output, dtype=np.float32)
    y = np.asarray(y, dtype=np.float32)
    A = np.asarray(A, dtype=np.float32)
    B_tl = np.asarray(B_tl, dtype=np.float32)
    beta_TL = np.asarray(beta_TL, dtype=np.float32)

    # wb: 9 beta-weight matrices then the ones matrix, each [128, 64]
    wb = np.zeros((P, (NSLAB + 1) * NG), dtype=np.float32)
    g_idx = np.arange(NG)
    for t in range(NSLAB):
        for s in range(NSLOT):
            wb[NSLOT * g_idx + s, NG * t + g_idx] = beta_TL[NSLAB * s + t]
    for s in range(NSLOT):
        wb[NSLOT * g_idx + s, NG * NSLAB + g_idx] = 1.0
    wb = wb.astype(ml_dtypes.bfloat16)

    def tcoef(arr):
        # [R, 18] -> partition 2g+s, free (chunk, slab t, col n), coef 9s+t
        return np.ascontiguousarray(
            arr.reshape(NG, NCHUNK, F, NSLOT, NSLAB)
            .transpose(0, 3, 1, 4, 2).reshape(P, NFILL * FILLW))

    def tfat(arr):
        # [R] -> [NFILL, 128(=64b+g), F]
        return arr.reshape(NG, NFILL, NBLK, F).transpose(1, 2, 0, 3) \
                  .reshape(NFILL, P, F)

    in_maps = []
    for i in range(N_CORES):
        lo, hi = i * R, min((i + 1) * R, N_TOTAL)
        mo_sh = np.zeros((R, NCOEF + 1), dtype=np.float32)
        mo_sh[:hi - lo] = model_output[lo:hi]
        a_sh = np.zeros((R, NCOEF), dtype=np.float32)
        a_sh[:hi - lo] = A[lo:hi]
        d_sh = mo_sh[:, NCOEF].copy()
        y_sh = np.zeros((R,), dtype=np.float32)
        y_sh[:hi - lo] = y[lo:hi, 0]
        b_sh = np.zeros((R,), dtype=np.float32)
        b_sh[:hi - lo] = B_tl[lo:hi, 0]

        a_t = tcoef(a_sh)
        c_t = tcoef(mo_sh[:, :NCOEF]).astype(ml_dtypes.float8_e4m3)
        dyb = np.ascontiguousarray(
            np.concatenate([tfat(d_sh), tfat(y_sh), tfat(b_sh)], axis=2)
            .transpose(1, 0, 2)
            .reshape(P, NFILL * 3 * F)).astype(ml_dtypes.bfloat16)
        m = {"c8": c_t, "dyb": dyb, "wb": wb}
        f8 = fp8_fills(xf8)
        f16 = [f for f in range(NFILL) if f not in f8]
        af = a_t.reshape(P, NFILL, FILLW)
        if f8:
            m["a8"] = np.ascontiguousarray(
                af[:, f8, :].reshape(P, -1)).astype(ml_dtypes.float8_e4m3)
        if f16:
            m["a16"] = np.ascontiguousarray(
                af[:, f16, :].reshape(P, -1)).astype(ml_dtypes.bfloat16)
        in_maps.append(m)
    return in_maps


def kernel(model_output, y, A, B_tl, beta_TL):
    nc = _build()
    in_maps = _prepare_in_maps(model_output, y, A, B_tl, beta_TL)
    res = run_bass_kernel_spmd(nc, in_maps, list(range(N_CORES)))
    total = 0.0
    for r in res.results:
        total += float(r["out"].astype(np.float64).sum())
    return np.asarray(total / N_TOTAL, dtype=np.float32)
